# revision 70
# baseline (speedup 1.0000x reference)
"""Trainium2 Bass kernel for nn_AttentionBlock (B=2, L=2048, D=1024, H=16).

Sharding: tensor-parallel over heads. Each of 8 cores computes 2 heads:
Wq/Wk/Wv column-sharded, Wo row-sharded; host sums the 8 partial outputs.

v2 structure (per core, bf16 datapath, fp32 PSUM):
  - v stored [token, head*64+d] per 128-token block (PE transpose, no ones col)
  - scores: 2 heads row-packed (K=64) in one PE slot
  - PV: 2 heads col-packed (M=64 at cols 0/64) in one PE slot
  - softmax denominators: ones[128,64] lhsT broadcast matmuls, col-packed,
    accumulating [128,LC] PSUM that directly IS the per-head-row denominator
    layout -> reciprocal_approx_fast -> normalize fused into the PSUM drain
  - projections interleaved into attention slots (work queue) so the PE and
    ACT engines stay dense; exp table preloaded via a dummy activation
"""
import numpy as np
from contextlib import ExitStack
from collections import deque

import concourse.bacc as bacc
import concourse.tile as tile
import concourse.mybir as mybir
from concourse import bass_utils
from concourse.masks import make_identity

F32 = mybir.dt.float32
F32R = mybir.dt.float32r
BF16 = mybir.dt.bfloat16
AF = mybir.ActivationFunctionType
ALU = mybir.AluOpType

B, L, D, H, DH = 2, 2048, 1024, 16, 64
NCORES = 8
HPC = H // NCORES       # heads per core = 2
DHC = HPC * DH          # 128
KT = D // 128           # 8 k-tiles over the contraction dim


def build(Lb=L, debug=False):
    BLb = B * Lb
    NJT = Lb // 128            # key tiles per batch = 16
    LC = min(512, Lb)          # query-chunk width
    NLC = Lb // LC             # query chunks per batch = 4
    PC = min(512, BLb)         # projection chunk width
    NPC = BLb // PC            # projection chunks (global) = 8
    NTT = BLb // 128           # 128-token tiles (global) = 32

    nc = bacc.Bacc("TRN2", target_bir_lowering=False, debug=debug, num_devices=8)

    # host supplies SBUF-layout (partition-major, chunk-contiguous) arrays so
    # every DMA is contiguous per partition (fast descriptor gen + transfer)
    NPC_ = BLb // min(512, BLb)
    xT = nc.dram_tensor("xT", [128, NPC_, KT, min(512, BLb)], BF16,
                        kind="ExternalInput")
    wq = nc.dram_tensor("wq", [128, KT, DHC], BF16, kind="ExternalInput")
    wk = nc.dram_tensor("wk", [128, KT, DHC], BF16, kind="ExternalInput")
    wv = nc.dram_tensor("wv", [128, KT, DHC], BF16, kind="ExternalInput")
    wo = nc.dram_tensor("wo", [DHC, D], BF16, kind="ExternalInput")
    bq = nc.dram_tensor("bq", [DHC, 1], F32, kind="ExternalInput")
    bk = nc.dram_tensor("bk", [DHC, 1], F32, kind="ExternalInput")
    bv = nc.dram_tensor("bv", [DHC, 1], F32, kind="ExternalInput")
    out = nc.dram_tensor("out", [BLb, D], BF16, kind="ExternalOutput")

    xT_v = xT.ap()                                          # [128, NPC, KT, PC]
    wq_v = wq.ap()
    wk_v = wk.ap()
    wv_v = wv.ap()

    with tile.TileContext(nc) as tc, ExitStack() as ctx:
        # --- pools ---
        persist = ctx.enter_context(tc.tile_pool(name="persist", bufs=1))
        xpool = ctx.enter_context(tc.tile_pool(name="xchunk", bufs=3))
        vstage = ctx.enter_context(tc.tile_pool(name="vstage", bufs=2))
        expool = ctx.enter_context(tc.tile_pool(name="expool", bufs=4))
        drpool = ctx.enter_context(tc.tile_pool(name="drpool", bufs=2))
        outpool = ctx.enter_context(tc.tile_pool(name="outpool", bufs=3))
        # PSUM: sc 2banks x2 + aT 1 + den 1 + single 1x2 = 8 banks
        scpool = ctx.enter_context(tc.tile_pool(name="scpool", bufs=2, space="PSUM"))
        accpool = ctx.enter_context(tc.tile_pool(name="accpool", bufs=1, space="PSUM"))
        psing = ctx.enter_context(tc.tile_pool(name="psing", bufs=2, space="PSUM"))

        # --- persistent tiles ---
        qT_sb = persist.tile([128, BLb], BF16, tag="qT")
        kT_sb = persist.tile([128, BLb], BF16, tag="kT")
        v_sb = persist.tile([128, NTT, 2, 65], F32R, tag="v")
        aT_sb = [
            persist.tile([128, Lb], BF16, tag=f"aT{b}", name=f"aT{b}")
            for b in range(B)
        ]
        wq_sb = persist.tile([128, KT, DHC], BF16, tag="wq")
        wk_sb = persist.tile([128, KT, DHC], BF16, tag="wk")
        wv_sb = persist.tile([128, KT, DHC], BF16, tag="wv")
        wo_sb = persist.tile([DHC, D], BF16, tag="wo")
        bq_sb = persist.tile([DHC, 1], F32, tag="bq")
        bk_sb = persist.tile([DHC, 1], F32, tag="bk")
        bv_sb = persist.tile([DHC, 1], F32, tag="bv")
        ident = persist.tile([128, 128], BF16, tag="ident")
        ones64 = persist.tile([65, DH], BF16, tag="ones64")   # row 64 used
        dumm = persist.tile([1, 2], F32, tag="dumm")

        xt_tiles = {}
        vt_tiles = {}

        # --- prologue: weight loads, constants, ACT exp-table preload ---
        # wk/x0 split in halves so the first k-col matmuls start as soon as
        # the first half lands (~2.5us) instead of after the full loads
        nc.sync.dma_start(wk_sb[:, 0:KT // 2, :], wk_v[:, 0:KT // 2, :])
        nc.sync.dma_start(wk_sb[:, KT // 2:, :], wk_v[:, KT // 2:, :])
        nc.sync.dma_start(wv_sb[:], wv_v)
        nc.sync.dma_start(wq_sb[:], wq_v)
        make_identity(nc, ident[:])
        nc.vector.memset(ones64[:], 1.0)
        nc.vector.memset(dumm[:], 0.0)
        # ones-augment columns of v (denominator trick)
        nc.vector.memset(v_sb[:, :, :, 64:65].bitcast(F32), 1.0)

        # --- projection work items ---
        def x_load(chn, split=False):
            def f():
                xt = xpool.tile([128, KT, PC], BF16, tag="xt")
                # scalar-engine HWDGE queue: don't serialize behind the
                # sync-engine weight/output DMA issue stream
                if split:
                    nc.scalar.dma_start(
                        xt[:, 0:KT // 2, :], xT_v[:, chn, 0:KT // 2, :]
                    )
                    nc.scalar.dma_start(
                        xt[:, KT // 2:, :], xT_v[:, chn, KT // 2:, :]
                    )
                else:
                    nc.scalar.dma_start(xt[:], xT_v[:, chn, :, :])
                xt_tiles[chn] = xt
            return f

        ps_tiles = {}

        def proj_col(chn, which, half=None):
            # half=None: full 8-kt column; half=0/1: split into two 4-kt
            # bursts (smaller PE bursts keep ACT fed between slots)
            def f():
                xt = xt_tiles[chn]
                w_sb, b_sb = {
                    "q": (wq_sb, bq_sb), "k": (wk_sb, bk_sb), "v": (wv_sb, bv_sb)
                }[which]
                if half in (None, 0):
                    ps_tiles[(chn, which)] = psing.tile(
                        [128, PC], F32, tag="single", name=f"ps_{chn}_{which}"
                    )
                ps = ps_tiles[(chn, which)]
                kts = range(KT) if half is None else \
                    range(half * KT // 2, (half + 1) * KT // 2)
                for kt in kts:
                    nc.tensor.matmul(
                        ps[:, :], w_sb[:, kt, :], xt[:, kt, :],
                        start=(kt == 0), stop=(kt == KT - 1),
                    )
                if half == 0:
                    return
                if which == "q":
                    nc.vector.tensor_scalar(
                        qT_sb[:, chn * PC:(chn + 1) * PC], ps[:, :],
                        b_sb[:, 0:1], None, ALU.add,
                    )
                elif which == "k":
                    nc.vector.tensor_scalar(
                        kT_sb[:, chn * PC:(chn + 1) * PC], ps[:, :],
                        b_sb[:, 0:1], None, ALU.add,
                    )
                else:
                    vt = vstage.tile([128, PC], BF16, tag="vt")
                    nc.vector.tensor_scalar(
                        vt[:], ps[:, :], b_sb[:, 0:1], None, ALU.add
                    )
                    vt_tiles[chn] = vt
            return f

        def v_transp(chn, jls):
            def f():
                vt = vt_tiles[chn]
                for jl in jls:
                    idx = chn * (PC // 128) + jl     # global 128-token tile
                    pt = psing.tile(
                        [128, 128], BF16, tag="single", padded_shape=[128, 512]
                    )
                    nc.tensor.transpose(
                        pt[:, :], vt[:, jl * 128:(jl + 1) * 128], ident[:]
                    )
                    nc.vector.tensor_copy(
                        v_sb[:, idx, :, 0:DH],
                        pt[:].rearrange("p (h c) -> p h c", h=2),
                    )
            return f

        def chunk_items(chn, with_q=True):
            items = [proj_col(chn, "k"), proj_col(chn, "v"),
                     v_transp(chn, (0, 1)), v_transp(chn, (2, 3))]
            if with_q:
                items.append(proj_col(chn, "q"))
            return items

        def chunk_items_split(chn):
            # smaller bursts for the non-deadline-critical b1 chunks
            return [proj_col(chn, "k", 0), proj_col(chn, "k", 1),
                    proj_col(chn, "v", 0), proj_col(chn, "v", 1),
                    v_transp(chn, (0, 1)), v_transp(chn, (2, 3)),
                    proj_col(chn, "q", 0), proj_col(chn, "q", 1)]

        # chunk 0 k/q/v inline (before attention starts). transposes of
        # chunk 0 go at the queue front (PV needs them only from jtx==2 on)
        # so the PE isn't gated on the v-bias DVE op.
        x_load(0, split=True)()
        x_load(1)()
        # exp-table preload AFTER the x issues (same scalar queue: the
        # ~1.3us table load must not delay the x0 transfer start)
        nc.scalar.activation(dumm[:], dumm[:], AF.Exp)
        nc.sync.dma_start(bq_sb[:], bq.ap())
        nc.sync.dma_start(bk_sb[:], bk.ap())
        nc.sync.dma_start(bv_sb[:], bv.ap())
        nc.sync.dma_start(wo_sb[:], wo.ap())
        proj_col(0, "k")()
        proj_col(0, "q")()
        proj_col(0, "v")()

        # queue: b0 chunks 1-3 early (needed inside b0 attention; x prefetched
        # at the front so transfers hide), then deferred q-cols, then all b1
        # chunks as small bursts (needed by slot 64).
        queue = deque()
        queue.append(v_transp(0, (0, 1)))
        queue.append(v_transp(0, (2, 3)))
        queue.append(x_load(2))
        queue.extend(chunk_items(1, with_q=False))
        queue.append(x_load(3))
        queue.extend(chunk_items(2, with_q=False))
        queue.extend(chunk_items(3, with_q=False))
        queue.append(proj_col(1, "q"))
        queue.append(proj_col(2, "q"))
        b0_items = len(queue)
        queue.append(proj_col(3, "q"))
        for chn in range(4, NPC):
            queue.append(x_load(chn))
            queue.extend(chunk_items_split(chn))
        b1_items = len(queue) - b0_items

        def pop_queue():
            if queue:
                queue.popleft()()

        # --- attention ---
        def make_drain1(b, lc, aT_ps):
            # part a (jtx==0): copy denominators out of PSUM (DVE)
            # part b (jtx==2): broadcast -> reciprocal -> fused normalize
            # The two-slot stagger hides the DVE copy latency from the PE.
            st = {}

            def fa():
                den = drpool.tile([65, HPC, LC], BF16, tag="den")
                nc.vector.tensor_copy(den[64:65, :, :], aT_ps[64:65, :, :])
                st["den"] = den

            def fb():
                den = st["den"]
                rep = psing.tile([128, LC], F32, tag="single")
                for h in range(HPC):
                    nc.tensor.matmul(
                        rep[h * DH:(h + 1) * DH, :],
                        ones64[64:65, :],
                        den[64:65, h, :],
                        start=True, stop=True,
                        tile_position=(64, h * DH),
                    )
                rrec = drpool.tile([128, LC], F32, tag="rrec")
                nc.vector.reciprocal_approx_fast(rrec[:, :], rep[:, :])
                for h in range(HPC):
                    nc.vector.tensor_mul(
                        aT_sb[b][h * DH:(h + 1) * DH, lc * LC:lc * LC + LC],
                        aT_ps[0:DH, h, :],
                        rrec[h * DH:(h + 1) * DH, :],
                    )
            return fa, fb

        def make_drain2(b, lc, t):
            # output projection for one 128-token tile of this query chunk
            def f():
                tt = lc * (LC // 128) + t
                ot = outpool.tile([128, D], BF16, tag="ot")
                for nch in range(2):
                    po = psing.tile([128, 512], F32, tag="single")
                    nc.tensor.matmul(
                        po[:, :],
                        aT_sb[b][:, tt * 128:(tt + 1) * 128],
                        wo_sb[:, nch * 512:(nch + 1) * 512],
                        start=True, stop=True,
                    )
                    nc.vector.tensor_copy(
                        ot[:, nch * 512:(nch + 1) * 512], po[:, :]
                    )
                nc.sync.dma_start(
                    out.ap()[b * Lb + tt * 128:b * Lb + (tt + 1) * 128, :],
                    ot[:],
                )
            return f

        # software pipeline: scores/exp run PIPE jt-steps ahead of PV, so the
        # previous chunk's drain (spread over jtx 2..10) never starves ACT.
        PIPE = 3
        drain1a = drain1b = None
        drain2_pending = []
        slot = 0
        for b in range(B):
            for lc in range(NLC):
                q0 = b * Lb + lc * LC
                aT_ps = accpool.tile(
                    [65, HPC, LC], F32, tag="acc", padded_shape=[65, HPC, 512]
                )
                ex_fifo = deque()
                for jtx in range(NJT + PIPE):
                    if jtx < NJT:
                        # consume projection work every slot, except when a
                        # drain piece already adds PE work to this slot
                        drain_here = (jtx == 2 and drain1b is not None) or \
                                     (jtx in (4, 6, 8, 10) and drain2_pending)
                        if not drain_here:
                            pop_queue()
                            if slot < 2:
                                pop_queue()   # front-load chunk-0 transposes
                        slot += 1
                    if jtx == 0 and drain1a is not None:
                        drain1a()
                        drain1a = None
                    if jtx == 2 and drain1b is not None:
                        drain1b()
                        drain1b = None
                    if jtx in (4, 6, 8, 10) and drain2_pending:
                        drain2_pending.pop(0)()
                    # interleave score and PV matmuls per head: the row-packed
                    # score pair shares rhs stream bandwidth anyway (~585ns),
                    # so solo-pipelined alternation wastes nothing and keeps
                    # each matmul back-to-back with the previous one
                    ex_prev = ex_fifo.popleft() if jtx >= PIPE else None
                    jt = jtx - PIPE
                    tt = b * NJT + jt                 # global 128-token tile
                    sc = None
                    if jtx < NJT:
                        k0 = b * Lb + jtx * 128
                        sc = scpool.tile(
                            [128, HPC, LC], F32, tag="sc",
                            padded_shape=[128, HPC, 512],
                        )
                    for h in range(HPC):
                        if sc is not None:
                            nc.tensor.matmul(
                                sc[:, h, :],
                                kT_sb[h * DH:(h + 1) * DH, k0:k0 + 128],
                                qT_sb[h * DH:(h + 1) * DH, q0:q0 + LC],
                                start=True, stop=True,
                                tile_position=(h * DH, 0),
                            )
                        if ex_prev is not None:
                            nc.tensor.matmul(
                                aT_ps[:, h, :],
                                v_sb[:, tt, h, :],
                                ex_prev[:, h, :],
                                start=(jt == 0), stop=(jt == NJT - 1),
                            )
                    if sc is not None:
                        ex = expool.tile([128, HPC, LC], F32R, tag="ex")
                        nc.scalar.activation(ex[:], sc[:], AF.Exp)
                        ex_fifo.append(ex)
                drain1a, drain1b = make_drain1(b, lc, aT_ps)
                drain2_pending = [make_drain2(b, lc, t) for t in range(LC // 128)]
        drain1a()
        drain1b()
        for f in drain2_pending:
            f()
        while queue:
            pop_queue()

    nc.compile()
    return nc


_NC_CACHE = {}


def _get_nc(Lb=L):
    if Lb not in _NC_CACHE:
        _NC_CACHE[Lb] = build(Lb)
    return _NC_CACHE[Lb]


def make_in_maps(x, Wq, bq, Wk, bk, Wv, bv, Wo, bo, Lb=L):
    import ml_dtypes
    bf16 = ml_dtypes.bfloat16
    s = np.float32(DH ** (-0.25))
    BLb = B * Lb
    PC = min(512, BLb)
    NPC = BLb // PC
    # [128, NPC, KT, PC]: partition-major, chunk-contiguous
    xT = np.ascontiguousarray(
        np.asarray(x, np.float32).reshape(NPC, PC, KT, 128)
        .transpose(3, 0, 2, 1)
    ).astype(bf16)

    def wprep(w):   # [D, DHC] -> [128, KT, DHC]
        return np.ascontiguousarray(
            w.reshape(KT, 128, -1).transpose(1, 0, 2).astype(bf16)
        )

    Wq, Wk, Wv, Wo = (np.asarray(a, np.float32) for a in (Wq, Wk, Wv, Wo))
    bq, bk, bv = (np.asarray(a, np.float32) for a in (bq, bk, bv))
    in_maps = []
    for c in range(NCORES):
        hs = slice(c * DHC, (c + 1) * DHC)
        in_maps.append({
            "xT": xT,
            "wq": wprep(Wq[:, hs] * s),
            "wk": wprep(Wk[:, hs] * s),
            "wv": wprep(Wv[:, hs]),
            "wo": np.ascontiguousarray(Wo[hs, :].astype(bf16)),
            "bq": np.ascontiguousarray((bq[hs] * s).reshape(DHC, 1)),
            "bk": np.ascontiguousarray((bk[hs] * s).reshape(DHC, 1)),
            "bv": np.ascontiguousarray(bv[hs].reshape(DHC, 1)),
        })
    return in_maps


def kernel(x, Wq, bq, Wk, bk, Wv, bv, Wo, bo, **run_kwargs):
    x = np.asarray(x, np.float32)
    nc = _get_nc(L)
    in_maps = make_in_maps(x, Wq, bq, Wk, bk, Wv, bv, Wo, bo, L)
    res = bass_utils.run_bass_kernel_spmd(nc, in_maps, list(range(NCORES)), **run_kwargs)
    acc = np.zeros((B * L, D), np.float32)
    for r in res.results:
        acc += np.asarray(r["out"], np.float32)
    acc += np.asarray(bo, np.float32)[None, :]
    out = acc.reshape(B, L, D)
    kernel.last_results = res
    return out
